# revision 12
# baseline (speedup 1.0000x reference)
"""GPT-2 style attention block (B=2, S=2048, D=1024, H=16) on 8 TRN2 NeuronCores.

Sharding: tensor-parallel over heads + data-parallel over batch.
Cores 0-3 handle batch 0, cores 4-7 handle batch 1; each core owns 4 of the
16 heads (its 256-column slice of the qkv projection and the matching
256-row slice of c_proj_w). Each core produces a partial output
[S, D] (stored fp16) = ctx_heads @ c_proj_rows; the 4 partials per batch
are summed on the host.

v3 design — hs ships from the host already transposed to [D, S] fp16
(layout prep, zero host FLOPs; device-side it used to cost 128 PE
transposes + 16 DVE copies + 17 ACT casts + 2x the DMA bytes), and the
projections are interleaved INTO the attention stream (NTFF traces of the
serial structure showed a PE-only projection phase followed by an
ACT-exp-paced attention phase; the engines complement, so overlap them):
  A. DMA hsT chunk s0-511, wqk ct0/ct2 slices -> QK projection for
     (Q hp0, K hp0) x s0-511 -> first scores at ~12us.
  B. attention qb0,qb1 with a filler queue drained 2 units/iter between
     each score+exp and its AV: vproj rt0-3, remaining qkproj units
     (hp1, then s512-1023, then ntb1), vproj rt4-15.
     PSUM: cx 2x2 banks, scp1 1x2, fill 2x1 = 8.
  C. attention qb3 then qb2 (scores/outproj share a 2x2-bank ring) with
     outproj units (qb0,qb1 under qb3; qb3 under qb2; qb2 at the tail)
     drained 1/iter.
Exact causal trimming, fp16 dataflow, ones-column softmax denominator,
approx-reciprocal normalize (fp32 custom DVE op), qkT bias-add on DVE,
outproj PSUM->SBUF on DVE; ACT runs exp only.

The bias rows (c_attn_b v-slice folded through c_proj_w, plus c_proj_b)
are added on the host during unsharding (exactly zero for the reference
setup_inputs). The causal_mask input is the deterministic tril mask from
setup_inputs(); causality is implemented analytically on device.
"""

import numpy as np

B, S, D, H = 2, 2048, 1024, 16
HD = D // H  # 64
N_CORES = 8
HPC = 4  # heads per core
GROUPS = 4  # cores per batch
HSL = HPC * HD  # 256: per-core head-column width

NDT = D // 128  # 8 contraction tiles
NRT = S // 128  # 16 row tiles
NQB = S // 512  # 4 query blocks

_nc_cache = {}


def _build():
    from collections import deque

    import concourse.bacc as bacc
    import concourse.mybir as mybir
    import concourse.tile as tile
    from concourse.masks import make_upper_triangular

    f32 = mybir.dt.float32
    f16 = mybir.dt.float16

    nc = bacc.Bacc("TRN2", debug=False, num_devices=N_CORES)

    hsT_d = nc.dram_tensor("hsT", [D, S], f16, kind="ExternalInput")
    wqk = nc.dram_tensor("wqk", [D, 2 * HSL], f16, kind="ExternalInput")
    wv = nc.dram_tensor("wv", [D, HSL], f16, kind="ExternalInput")
    wp = nc.dram_tensor("wp", [HSL, D], f16, kind="ExternalInput")
    bqk = nc.dram_tensor("bqk", [2 * HSL], f32, kind="ExternalInput")
    outp = nc.dram_tensor("outp", [S, D], f16, kind="ExternalOutput")

    with tile.TileContext(nc) as tc:
        with (
            tc.tile_pool(name="persist", bufs=1) as persist,
            tc.tile_pool(name="es", bufs=10) as es_pool,
            tc.tile_pool(name="rb", bufs=4) as rb_pool,
            tc.tile_pool(name="ob", bufs=6) as ob_pool,
        ):
            # ---- persistent SBUF ----
            hsT = persist.tile([128, NDT, S], f16)  # [d%128, d//128, s]
            qkT = persist.tile([128, 4, S], f16)  # [Q hp0|Q hp1|K hp0|K hp1]
            vv = persist.tile([128, NRT, HPC * (HD + 1)], f16)  # V aug
            wqk_sb = persist.tile([128, NDT, 2 * HSL], f16)
            wv_sb = persist.tile([128, NDT, HSL], f16)
            wp_sb = persist.tile([128, 2, D], f16)
            bqk_sb = persist.tile([128, 4], f32)
            dmask = persist.tile([128, 128], f16)  # 1 where q(col) >= k(row)
            ctxT = persist.tile([128, 2, S], f16)

            make_upper_triangular(nc, dmask, val=1.0, diag=True)
            # ones columns of V_aug (data cols overwritten by vproj)
            nc.gpsimd.memset(vv, 1.0)

            hsT_src = hsT_d.rearrange("(t p) s -> p t s", p=128)
            wqk_src = wqk.rearrange("(t p) n -> p t n", p=128)

            # ---------- emitters ----------
            def emit_vproj(rt, pool, tag):
                pv = pool.tile([128, HSL], f32, tag=tag, name=f"pv{rt}")
                for dt in range(NDT):
                    nc.tensor.matmul(
                        pv,
                        hsT[:, dt, rt * 128 : (rt + 1) * 128],
                        wv_sb[:, dt, :],
                        start=(dt == 0),
                        stop=(dt == NDT - 1),
                    )
                vtgt = vv[:, rt, :].rearrange("p (h c) -> p h c", c=HD + 1)
                nc.vector.tensor_copy(
                    vtgt[:, :, 0:HD],
                    pv.rearrange("p (h c) -> p h c", c=HD),
                )

            def emit_qkproj(ct, sb, pool, tag):
                # one [128, 512] column-block of qkT for s-block sb
                pj = pool.tile(
                    [128, 512], f32, tag=tag, name=f"pj{ct}_{sb}"
                )
                for dt in range(NDT):
                    nc.tensor.matmul(
                        pj,
                        wqk_sb[:, dt, ct * 128 : (ct + 1) * 128],
                        hsT[:, dt, sb * 512 : (sb + 1) * 512],
                        start=(dt == 0),
                        stop=(dt == NDT - 1),
                    )
                nc.vector.tensor_scalar_add(
                    qkT[:, ct, sb * 512 : (sb + 1) * 512],
                    pj,
                    bqk_sb[:, ct : ct + 1],
                )

            def emit_scores_exp(qb, hp, kt, pool, tag):
                j = kt - 4 * qb
                w = 512 if j < 0 else 512 - 128 * j
                qo = 512 - w
                scp = pool.tile(
                    [128, 1024], f32, tag=tag, name=f"scp{qb}_{hp}_{kt}"
                )
                for hh in range(2):
                    nc.tensor.matmul(
                        scp[:, 512 * hh + qo : 512 * (hh + 1)],
                        qkT[
                            hh * 64 : (hh + 1) * 64,
                            2 + hp,
                            kt * 128 : (kt + 1) * 128,
                        ],
                        qkT[
                            hh * 64 : (hh + 1) * 64,
                            hp,
                            qb * 512 + qo : (qb + 1) * 512,
                        ],
                        start=True,
                        stop=True,
                        tile_position=(hh * 64, 0),
                    )
                es = es_pool.tile([128, 1024], f16, tag="es", name="es")
                scp3 = scp.rearrange("p (h c) -> p h c", c=512)
                es3 = es.rearrange("p (h c) -> p h c", c=512)
                nc.scalar.activation(
                    es3[:, :, qo:512],
                    scp3[:, :, qo:512],
                    mybir.ActivationFunctionType.Exp,
                    scale=float(1.0 / np.sqrt(HD)),
                )
                if j >= 0:
                    for hh in range(2):
                        nc.vector.tensor_mul(
                            es[:, 512 * hh + qo : 512 * hh + qo + 128],
                            es[:, 512 * hh + qo : 512 * hh + qo + 128],
                            dmask,
                        )
                return es

            def emit_av(qb, hp, kt, cx, es):
                j = kt - 4 * qb
                w = 512 if j < 0 else 512 - 128 * j
                qo = 512 - w
                kmax = 4 * (qb + 1)
                for hh in range(2):
                    h = 2 * hp + hh
                    nc.tensor.matmul(
                        cx[:, hh, qo:512],
                        vv[:, kt, h * (HD + 1) : (h + 1) * (HD + 1)],
                        es[:, 512 * hh + qo : 512 * (hh + 1)],
                        start=(kt == 0),
                        stop=(kt == kmax - 1),
                    )

            def emit_normalize(qb, hp, cx):
                # row 64 of cx holds the softmax denominator
                denf = rb_pool.tile([1, 1024], f32, tag="denf", name="denf")
                denf3 = denf.rearrange("p (h c) -> p h c", c=512)
                nc.vector.tensor_copy(denf3, cx[64:65, :, :])
                recf = rb_pool.tile([1, 1024], f32, tag="recf", name="recf")
                recf3 = recf.rearrange("p (h c) -> p h c", c=512)
                for hh in range(2):
                    nc.vector.reciprocal_approx_fast(
                        recf3[:, hh, :], denf3[:, hh, :]
                    )
                rec = rb_pool.tile([1, 1024], f16, tag="rec", name="rec")
                rec3 = rec.rearrange("p (h c) -> p h c", c=512)
                nc.vector.tensor_copy(rec, recf)
                rbt = rb_pool.tile([64, 1024], f16, tag="rbt", name="rbt")
                rbt3 = rbt.rearrange("p (h c) -> p h c", c=512)
                for hh in range(2):
                    nc.gpsimd.partition_broadcast(
                        rbt3[:, hh, :], rec3[:, hh, :]
                    )
                    nc.vector.tensor_mul(
                        ctxT[
                            hh * 64 : hh * 64 + 64,
                            hp,
                            qb * 512 : (qb + 1) * 512,
                        ],
                        cx[0:64, hh, :],
                        rbt3[:, hh, :],
                    )

            def emit_outproj_half(mt, half, pool, tag):
                po = pool.tile(
                    [128, 512], f32, tag=tag, name=f"poh{mt}_{half}"
                )
                for ht in range(2):
                    nc.tensor.matmul(
                        po,
                        ctxT[:, ht, mt * 128 : (mt + 1) * 128],
                        wp_sb[:, ht, half * 512 : (half + 1) * 512],
                        start=(ht == 0),
                        stop=(ht == 1),
                    )
                ob = ob_pool.tile([128, 512], f16, tag="ob", name="ob")
                nc.vector.tensor_copy(ob, po)
                nc.sync.dma_start(
                    out=outp[
                        mt * 128 : (mt + 1) * 128,
                        half * 512 : (half + 1) * 512,
                    ],
                    in_=ob,
                )

            def emit_outproj_mt(mt, pool, tag):
                po = pool.tile([128, 1024], f32, tag=tag, name=f"po{mt}")
                for ht in range(2):
                    for half in range(2):
                        nc.tensor.matmul(
                            po[:, half * 512 : (half + 1) * 512],
                            ctxT[:, ht, mt * 128 : (mt + 1) * 128],
                            wp_sb[:, ht, half * 512 : (half + 1) * 512],
                            start=(ht == 0),
                            stop=(ht == 1),
                        )
                ob = ob_pool.tile([128, 1024], f16, tag="ob", name="ob")
                nc.vector.tensor_copy(ob, po)
                nc.sync.dma_start(
                    out=outp[mt * 128 : (mt + 1) * 128, :], in_=ob
                )

            # ---------- DMA issue (single FIFO queue -> priority order) --
            # wqk ct0/ct2 slices + bqk + hsT s0-511 first: the first QK
            # projections and scores gate everything else
            for ct in (0, 2):
                nc.sync.dma_start(
                    out=wqk_sb[:, :, ct * 128 : (ct + 1) * 128],
                    in_=wqk_src[:, :, ct * 128 : (ct + 1) * 128],
                )
            nc.sync.dma_start(
                out=bqk_sb, in_=bqk.rearrange("(t p) -> p t", p=128)
            )
            nc.sync.dma_start(
                out=hsT[:, :, 0:512], in_=hsT_src[:, :, 0:512]
            )
            nc.sync.dma_start(
                out=wv_sb, in_=wv.rearrange("(t p) n -> p t n", p=128)
            )
            nc.sync.dma_start(
                out=hsT[:, :, 512:1024], in_=hsT_src[:, :, 512:1024]
            )
            for ct in (1, 3):
                nc.sync.dma_start(
                    out=wqk_sb[:, :, ct * 128 : (ct + 1) * 128],
                    in_=wqk_src[:, :, ct * 128 : (ct + 1) * 128],
                )
            nc.sync.dma_start(
                out=hsT[:, :, 1024:2048], in_=hsT_src[:, :, 1024:2048]
            )
            nc.sync.dma_start(
                out=wp_sb, in_=wp.rearrange("(t p) n -> p t n", p=128)
            )

            # ---------- stage A: earliest QK projections ----------
            pjB_ctx = tc.tile_pool(name="pjB", bufs=2, space="PSUM")
            pjB = pjB_ctx.__enter__()
            for ct in (0, 2):
                emit_qkproj(ct, 0, pjB, "pj")
            pjB_ctx.__exit__(None, None, None)

            cx_ctx = tc.tile_pool(name="cx", bufs=2, space="PSUM")
            cx_pool = cx_ctx.__enter__()
            scp1_ctx = tc.tile_pool(name="scp1", bufs=1, space="PSUM")
            scp1 = scp1_ctx.__enter__()
            fill_ctx = tc.tile_pool(name="fill", bufs=2, space="PSUM")
            fill = fill_ctx.__enter__()

            # ---------- filler queue for stage B ----------
            fillers = deque()

            def drain(n):
                for _ in range(n):
                    if fillers:
                        fillers.popleft()()

            def q_vproj(dq, rts):
                for rt in rts:
                    dq.append(lambda rt=rt: emit_vproj(rt, fill, "fill"))

            def q_qkproj(dq, units):
                for ct, sb in units:
                    dq.append(
                        lambda ct=ct, sb=sb: emit_qkproj(ct, sb, fill, "fill")
                    )

            q_vproj(fillers, (0, 1, 2, 3))
            q_qkproj(fillers, [(1, 0), (3, 0)])  # qb0-hp1
            q_qkproj(fillers, [(0, 1), (2, 1)])  # qb1-hp0
            q_vproj(fillers, (4, 5, 6, 7))  # qb1 kt4-7
            q_qkproj(fillers, [(1, 1), (3, 1)])  # qb1-hp1
            q_qkproj(fillers, [(0, 3), (2, 3)])  # qb3-hp0 Q / K kt12-15
            # (1,3) queued after qb0's outproj units below

            # ---------- stage B: attention qb0, qb1 + fillers ----------
            for qb in (0, 1):
                for hp in range(2):
                    cx = cx_pool.tile(
                        [65, 2, 512], f32, tag="cx", name=f"cx{qb}_{hp}"
                    )
                    for kt in range(4 * (qb + 1)):
                        es = emit_scores_exp(qb, hp, kt, scp1, "scp")
                        drain(2)
                        emit_av(qb, hp, kt, cx, es)
                    emit_normalize(qb, hp, cx)
                if qb == 0:
                    # qb0 done: its outproj halves become filler work
                    for mt in range(4):
                        for half in range(2):
                            fillers.append(
                                lambda mt=mt, half=half: emit_outproj_half(
                                    mt, half, fill, "fill"
                                )
                            )
                    q_qkproj(fillers, [(1, 3)])  # qb3-hp1 Q
            while fillers:
                fillers.popleft()()

            # ---------- stage C: attention qb3, qb2 + deferred work -----
            fill_ctx.__exit__(None, None, None)
            scp1_ctx.__exit__(None, None, None)
            big_ctx = tc.tile_pool(name="big", bufs=2, space="PSUM")
            big = big_ctx.__enter__()

            deferred = deque()
            deferred.append(lambda: emit_qkproj(2, 2, big, "big"))
            for rt in (8, 9, 10, 11):
                deferred.append(lambda rt=rt: emit_vproj(rt, big, "big"))
            for mt in range(4, 8):  # qb1 outproj
                deferred.append(
                    lambda mt=mt: emit_outproj_mt(mt, big, "big")
                )
            deferred.append(lambda: emit_qkproj(3, 2, big, "big"))
            for rt in (12, 13, 14, 15):
                deferred.append(lambda rt=rt: emit_vproj(rt, big, "big"))
            deferred.append(lambda: emit_qkproj(3, 3, big, "big"))
            deferred.append(lambda: emit_qkproj(0, 2, big, "big"))
            deferred.append(lambda: emit_qkproj(1, 2, big, "big"))

            for qb in (3, 2):
                for hp in range(2):
                    cx = cx_pool.tile(
                        [65, 2, 512], f32, tag="cx", name=f"cx{qb}_{hp}"
                    )
                    es_prev = None
                    for kt in range(4 * (qb + 1)):
                        es = emit_scores_exp(qb, hp, kt, big, "big")
                        if es_prev is not None:
                            emit_av(qb, hp, kt - 1, cx, es_prev)
                        if deferred:
                            deferred.popleft()()
                        es_prev = es
                    emit_av(qb, hp, 4 * (qb + 1) - 1, cx, es_prev)
                    emit_normalize(qb, hp, cx)
                for mt in range(4 * qb, 4 * qb + 4):
                    deferred.append(
                        lambda mt=mt: emit_outproj_mt(mt, big, "big")
                    )
            while deferred:
                deferred.popleft()()

            big_ctx.__exit__(None, None, None)
            cx_ctx.__exit__(None, None, None)

    nc.compile()
    return nc


def build_kernel(matmul_dtype=None, av_dtype=None):
    # single fp16 variant; dtype args accepted for harness compat
    if "k" not in _nc_cache:
        _nc_cache["k"] = _build()
    return _nc_cache["k"]


def make_in_maps(
    hidden_states, c_attn_w, c_attn_b, c_proj_w, c_proj_b,
    matmul_dtype=None, av_dtype=None,
):
    hidden_states = np.asarray(hidden_states, dtype=np.float32)
    c_attn_w = np.asarray(c_attn_w, dtype=np.float32)
    c_attn_b = np.asarray(c_attn_b, dtype=np.float32)
    c_proj_w = np.asarray(c_proj_w, dtype=np.float32)
    c_proj_b = np.asarray(c_proj_b, dtype=np.float32)

    in_maps = []
    for c in range(N_CORES):
        b, g = divmod(c, GROUPS)
        cs = slice(g * HSL, (g + 1) * HSL)
        wq = c_attn_w[:, g * HSL : (g + 1) * HSL]
        wk = c_attn_w[:, D + g * HSL : D + (g + 1) * HSL]
        wvs = c_attn_w[:, 2 * D + g * HSL : 2 * D + (g + 1) * HSL]
        bq = c_attn_b[g * HSL : (g + 1) * HSL]
        bk = c_attn_b[D + g * HSL : D + (g + 1) * HSL]
        bv = c_attn_b[2 * D + g * HSL : 2 * D + (g + 1) * HSL]
        wps = c_proj_w[cs, :]
        rr = bv.astype(np.float64) @ wps.astype(np.float64)
        if g == 0:
            rr = rr + c_proj_b
        in_maps.append(
            {
                "hsT": np.ascontiguousarray(
                    hidden_states[b].T.astype(np.float16)
                ),
                "wqk": np.ascontiguousarray(
                    np.concatenate([wq, wk], axis=1).astype(np.float16)
                ),
                "wv": np.ascontiguousarray(wvs.astype(np.float16)),
                "wp": np.ascontiguousarray(wps.astype(np.float16)),
                "bqk": np.ascontiguousarray(np.concatenate([bq, bk])),
                "_rrow": np.ascontiguousarray(rr.astype(np.float32)),
            }
        )
    return in_maps


def kernel(
    hidden_states,
    c_attn_w,
    c_attn_b,
    c_proj_w,
    c_proj_b,
    causal_mask=None,
    **_unused,
):
    from concourse.bass_utils import run_bass_kernel_spmd

    nc = build_kernel()
    in_maps = make_in_maps(
        hidden_states, c_attn_w, c_attn_b, c_proj_w, c_proj_b
    )
    rrows = [m.pop("_rrow") for m in in_maps]
    res = run_bass_kernel_spmd(nc, in_maps, list(range(N_CORES)))
    out = np.zeros((B, S, D), dtype=np.float32)
    for c in range(N_CORES):
        out[c // GROUPS] += res.results[c]["outp"].astype(np.float32)
        out[c // GROUPS] += rrows[c]
    return out


# revision 14
# speedup vs baseline: 1.0242x; 1.0242x over previous
"""GPT-2 style attention block (B=2, S=2048, D=1024, H=16) on 8 TRN2 NeuronCores.

Sharding: tensor-parallel over heads + data-parallel over batch.
Cores 0-3 handle batch 0, cores 4-7 handle batch 1; each core owns 4 of the
16 heads (its 256-column slice of the qkv projection and the matching
256-row slice of c_proj_w). Each core produces a partial output
[S, D] (stored fp16) = ctx_heads @ c_proj_rows; the 4 partials per batch
are summed on the host.

v3 design — hs ships from the host already transposed to [D, S] fp16
(layout prep, zero host FLOPs; device-side it used to cost 128 PE
transposes + 16 DVE copies + 17 ACT casts + 2x the DMA bytes), and the
projections are interleaved INTO the attention stream (NTFF traces of the
serial structure showed a PE-only projection phase followed by an
ACT-exp-paced attention phase; the engines complement, so overlap them):
  A. DMA hsT chunk s0-511, wqk ct0/ct2 slices -> QK projection for
     (Q hp0, K hp0) x s0-511 -> first scores at ~12us.
  B. attention qb0,qb1 with a filler queue drained 2 units/iter between
     each score+exp and its AV: vproj rt0-3, remaining qkproj units
     (hp1, then s512-1023, then ntb1), vproj rt4-15.
     PSUM: cx 2x2 banks, scp1 1x2, fill 2x1 = 8.
  C. attention qb3 then qb2 (scores/outproj share a 2x2-bank ring) with
     outproj units (qb0,qb1 under qb3; qb3 under qb2; qb2 at the tail)
     drained 1/iter.
Exact causal trimming, fp16 dataflow, ones-column softmax denominator,
approx-reciprocal normalize (fp32 custom DVE op), qkT bias-add on DVE,
outproj PSUM->SBUF on DVE; ACT runs exp only.

The bias rows (c_attn_b v-slice folded through c_proj_w, plus c_proj_b)
are added on the host during unsharding (exactly zero for the reference
setup_inputs). The causal_mask input is the deterministic tril mask from
setup_inputs(); causality is implemented analytically on device.
"""

import numpy as np

B, S, D, H = 2, 2048, 1024, 16
HD = D // H  # 64
N_CORES = 8
HPC = 4  # heads per core
GROUPS = 4  # cores per batch
HSL = HPC * HD  # 256: per-core head-column width

NDT = D // 128  # 8 contraction tiles
NRT = S // 128  # 16 row tiles
NQB = S // 512  # 4 query blocks
# wqk ships host-reordered as column blocks [ct0|ct2|ct1|ct3] so the two
# halves needed first (hp0's Q and K) are one contiguous DMA each
CT2COL = {0: 0, 2: 128, 1: 256, 3: 384}
CT2IDX = {0: 0, 2: 1, 1: 2, 3: 3}

_nc_cache = {}


def _build():
    from collections import deque

    import concourse.bacc as bacc
    import concourse.mybir as mybir
    import concourse.tile as tile
    from concourse.masks import make_upper_triangular

    f32 = mybir.dt.float32
    f16 = mybir.dt.float16

    nc = bacc.Bacc("TRN2", debug=False, num_devices=N_CORES)

    hsT_d = nc.dram_tensor("hsT", [D, S], f16, kind="ExternalInput")
    wqk = nc.dram_tensor("wqk", [D, 2 * HSL], f16, kind="ExternalInput")
    wv = nc.dram_tensor("wv", [D, HSL], f16, kind="ExternalInput")
    wp = nc.dram_tensor("wp", [HSL, D], f16, kind="ExternalInput")
    bqk = nc.dram_tensor("bqk", [2 * HSL], f32, kind="ExternalInput")
    outp = nc.dram_tensor("outp", [S, D], f16, kind="ExternalOutput")

    with tile.TileContext(nc) as tc:
        with (
            tc.tile_pool(name="persist", bufs=1) as persist,
            tc.tile_pool(name="es", bufs=10) as es_pool,
            tc.tile_pool(name="rb", bufs=4) as rb_pool,
            tc.tile_pool(name="ob", bufs=6) as ob_pool,
        ):
            # ---- persistent SBUF ----
            hsT = persist.tile([128, NDT, S], f16)  # [d%128, d//128, s]
            qkT = persist.tile([128, 4, S], f16)  # [Q hp0|Q hp1|K hp0|K hp1]
            vv = persist.tile([128, NRT, HPC * (HD + 1)], f16)  # V aug
            wqk_sb = persist.tile([128, NDT, 2 * HSL], f16)
            wv_sb = persist.tile([128, NDT, HSL], f16)
            wp_sb = persist.tile([128, 2, D], f16)
            bqk_sb = persist.tile([128, 4], f32)
            dmask = persist.tile([128, 128], f16)  # 1 where q(col) >= k(row)
            ctxT = persist.tile([128, 2, S], f16)

            make_upper_triangular(nc, dmask, val=1.0, diag=True)
            # ones columns of V_aug (data cols overwritten by vproj)
            nc.gpsimd.memset(vv, 1.0)

            hsT_src = hsT_d.rearrange("(t p) s -> p t s", p=128)
            wqk_src = wqk.rearrange("(t p) n -> p t n", p=128)

            # ---------- emitters ----------
            def emit_vproj(rt, pool, tag):
                pv = pool.tile([128, HSL], f32, tag=tag, name=f"pv{rt}")
                for dt in range(NDT):
                    nc.tensor.matmul(
                        pv,
                        hsT[:, dt, rt * 128 : (rt + 1) * 128],
                        wv_sb[:, dt, :],
                        start=(dt == 0),
                        stop=(dt == NDT - 1),
                    )
                vtgt = vv[:, rt, :].rearrange("p (h c) -> p h c", c=HD + 1)
                nc.vector.tensor_copy(
                    vtgt[:, :, 0:HD],
                    pv.rearrange("p (h c) -> p h c", c=HD),
                )

            def emit_qkproj(ct, sb, pool, tag):
                # one [128, 512] column-block of qkT for s-block sb
                pj = pool.tile(
                    [128, 512], f32, tag=tag, name=f"pj{ct}_{sb}"
                )
                co = CT2COL[ct]
                for dt in range(NDT):
                    nc.tensor.matmul(
                        pj,
                        wqk_sb[:, dt, co : co + 128],
                        hsT[:, dt, sb * 512 : (sb + 1) * 512],
                        start=(dt == 0),
                        stop=(dt == NDT - 1),
                    )
                nc.vector.tensor_scalar_add(
                    qkT[:, ct, sb * 512 : (sb + 1) * 512],
                    pj,
                    bqk_sb[:, CT2IDX[ct] : CT2IDX[ct] + 1],
                )

            def emit_scores_exp(qb, hp, kt, pool, tag):
                j = kt - 4 * qb
                w = 512 if j < 0 else 512 - 128 * j
                qo = 512 - w
                scp = pool.tile(
                    [128, 1024], f32, tag=tag, name=f"scp{qb}_{hp}_{kt}"
                )
                for hh in range(2):
                    nc.tensor.matmul(
                        scp[:, 512 * hh + qo : 512 * (hh + 1)],
                        qkT[
                            hh * 64 : (hh + 1) * 64,
                            2 + hp,
                            kt * 128 : (kt + 1) * 128,
                        ],
                        qkT[
                            hh * 64 : (hh + 1) * 64,
                            hp,
                            qb * 512 + qo : (qb + 1) * 512,
                        ],
                        start=True,
                        stop=True,
                        tile_position=(hh * 64, 0),
                    )
                es = es_pool.tile([128, 1024], f16, tag="es", name="es")
                scp3 = scp.rearrange("p (h c) -> p h c", c=512)
                es3 = es.rearrange("p (h c) -> p h c", c=512)
                nc.scalar.activation(
                    es3[:, :, qo:512],
                    scp3[:, :, qo:512],
                    mybir.ActivationFunctionType.Exp,
                    scale=float(1.0 / np.sqrt(HD)),
                )
                if j >= 0:
                    for hh in range(2):
                        nc.vector.tensor_mul(
                            es[:, 512 * hh + qo : 512 * hh + qo + 128],
                            es[:, 512 * hh + qo : 512 * hh + qo + 128],
                            dmask,
                        )
                return es

            def emit_av(qb, hp, kt, cx, es):
                j = kt - 4 * qb
                w = 512 if j < 0 else 512 - 128 * j
                qo = 512 - w
                kmax = 4 * (qb + 1)
                for hh in range(2):
                    h = 2 * hp + hh
                    nc.tensor.matmul(
                        cx[:, hh, qo:512],
                        vv[:, kt, h * (HD + 1) : (h + 1) * (HD + 1)],
                        es[:, 512 * hh + qo : 512 * (hh + 1)],
                        start=(kt == 0),
                        stop=(kt == kmax - 1),
                    )

            def emit_normalize(qb, hp, cx):
                # row 64 of cx holds the softmax denominator
                denf = rb_pool.tile([1, 1024], f32, tag="denf", name="denf")
                denf3 = denf.rearrange("p (h c) -> p h c", c=512)
                nc.vector.tensor_copy(denf3, cx[64:65, :, :])
                recf = rb_pool.tile([1, 1024], f32, tag="recf", name="recf")
                recf3 = recf.rearrange("p (h c) -> p h c", c=512)
                for hh in range(2):
                    nc.vector.reciprocal_approx_fast(
                        recf3[:, hh, :], denf3[:, hh, :]
                    )
                rec = rb_pool.tile([1, 1024], f16, tag="rec", name="rec")
                rec3 = rec.rearrange("p (h c) -> p h c", c=512)
                nc.vector.tensor_copy(rec, recf)
                rbt = rb_pool.tile([64, 1024], f16, tag="rbt", name="rbt")
                rbt3 = rbt.rearrange("p (h c) -> p h c", c=512)
                for hh in range(2):
                    nc.gpsimd.partition_broadcast(
                        rbt3[:, hh, :], rec3[:, hh, :]
                    )
                    nc.vector.tensor_mul(
                        ctxT[
                            hh * 64 : hh * 64 + 64,
                            hp,
                            qb * 512 : (qb + 1) * 512,
                        ],
                        cx[0:64, hh, :],
                        rbt3[:, hh, :],
                    )

            def emit_outproj_half(mt, half, pool, tag):
                po = pool.tile(
                    [128, 512], f32, tag=tag, name=f"poh{mt}_{half}"
                )
                for ht in range(2):
                    nc.tensor.matmul(
                        po,
                        ctxT[:, ht, mt * 128 : (mt + 1) * 128],
                        wp_sb[:, ht, half * 512 : (half + 1) * 512],
                        start=(ht == 0),
                        stop=(ht == 1),
                    )
                ob = ob_pool.tile([128, 512], f16, tag="ob", name="ob")
                nc.vector.tensor_copy(ob, po)
                nc.sync.dma_start(
                    out=outp[
                        mt * 128 : (mt + 1) * 128,
                        half * 512 : (half + 1) * 512,
                    ],
                    in_=ob,
                )

            def emit_outproj_mt(mt, pool, tag):
                po = pool.tile([128, 1024], f32, tag=tag, name=f"po{mt}")
                for ht in range(2):
                    for half in range(2):
                        nc.tensor.matmul(
                            po[:, half * 512 : (half + 1) * 512],
                            ctxT[:, ht, mt * 128 : (mt + 1) * 128],
                            wp_sb[:, ht, half * 512 : (half + 1) * 512],
                            start=(ht == 0),
                            stop=(ht == 1),
                        )
                ob = ob_pool.tile([128, 1024], f16, tag="ob", name="ob")
                nc.vector.tensor_copy(ob, po)
                nc.sync.dma_start(
                    out=outp[mt * 128 : (mt + 1) * 128, :], in_=ob
                )

            # ---------- DMA issue (single FIFO queue -> priority order) --
            # wqk first half (hp0's Q,K columns) + bqk + hsT s0-511 first:
            # the first QK projections and scores gate everything else
            nc.sync.dma_start(
                out=wqk_sb[:, :, 0:256], in_=wqk_src[:, :, 0:256]
            )
            nc.sync.dma_start(
                out=bqk_sb, in_=bqk.rearrange("(t p) -> p t", p=128)
            )
            nc.sync.dma_start(
                out=hsT[:, :, 0:512], in_=hsT_src[:, :, 0:512]
            )
            nc.sync.dma_start(
                out=wv_sb, in_=wv.rearrange("(t p) n -> p t n", p=128)
            )
            nc.sync.dma_start(
                out=hsT[:, :, 512:1024], in_=hsT_src[:, :, 512:1024]
            )
            nc.sync.dma_start(
                out=wqk_sb[:, :, 256:512], in_=wqk_src[:, :, 256:512]
            )
            nc.sync.dma_start(
                out=hsT[:, :, 1024:2048], in_=hsT_src[:, :, 1024:2048]
            )
            nc.sync.dma_start(
                out=wp_sb, in_=wp.rearrange("(t p) n -> p t n", p=128)
            )

            # ---------- stage A: earliest QK projections ----------
            pjB_ctx = tc.tile_pool(name="pjB", bufs=2, space="PSUM")
            pjB = pjB_ctx.__enter__()
            for ct in (0, 2):
                emit_qkproj(ct, 0, pjB, "pj")
            pjB_ctx.__exit__(None, None, None)

            cx_ctx = tc.tile_pool(name="cx", bufs=2, space="PSUM")
            cx_pool = cx_ctx.__enter__()
            scp1_ctx = tc.tile_pool(name="scp1", bufs=1, space="PSUM")
            scp1 = scp1_ctx.__enter__()
            fill_ctx = tc.tile_pool(name="fill", bufs=2, space="PSUM")
            fill = fill_ctx.__enter__()

            # ---------- filler queue for stage B ----------
            fillers = deque()

            def drain(n):
                for _ in range(n):
                    if fillers:
                        fillers.popleft()()

            def q_vproj(dq, pool, tag, rts):
                for rt in rts:
                    dq.append(
                        lambda rt=rt: emit_vproj(rt, pool, tag)
                    )

            def q_qkproj(dq, pool, tag, units):
                for ct, sb in units:
                    dq.append(
                        lambda ct=ct, sb=sb: emit_qkproj(ct, sb, pool, tag)
                    )

            q_vproj(fillers, fill, "fill", (0, 1, 2, 3))
            q_qkproj(fillers, fill, "fill", [(1, 0), (3, 0)])  # qb0-hp1
            q_qkproj(fillers, fill, "fill", [(0, 1), (2, 1)])  # qb1-hp0
            q_vproj(fillers, fill, "fill", (4, 5, 6, 7))  # qb1 kt4-7
            q_qkproj(fillers, fill, "fill", [(1, 1), (3, 1)])  # qb1-hp1
            q_qkproj(fillers, fill, "fill", [(0, 3), (2, 3)])  # qb3-hp0

            # ---------- stage B: attention qb0, qb1 + fillers ----------
            it = 0
            for qb in (0, 1):
                for hp in range(2):
                    cx = cx_pool.tile(
                        [65, 2, 512], f32, tag="cx", name=f"cx{qb}_{hp}"
                    )
                    for kt in range(4 * (qb + 1)):
                        es = emit_scores_exp(qb, hp, kt, scp1, "scp")
                        drain(2 if it < 4 else 1)
                        emit_av(qb, hp, kt, cx, es)
                        it += 1
                    emit_normalize(qb, hp, cx)
            while fillers:
                fillers.popleft()()

            # ---------- stage C: attention qb3, qb2 + deferred work -----
            # Deferred units are drained in PAIRS on odd kt so that scp
            # tiles keep alternating ring buffers (an odd number of
            # interleaved allocations would pin every scp to one buffer
            # and serialize score(k+1) behind exp(k)).
            fill_ctx.__exit__(None, None, None)
            scp1_ctx.__exit__(None, None, None)
            big_ctx = tc.tile_pool(name="big", bufs=2, space="PSUM")
            big = big_ctx.__enter__()

            deferred = deque()
            q_qkproj(deferred, big, "big", [(1, 3)])  # qb3-hp1 Q
            q_qkproj(deferred, big, "big", [(2, 2)])  # K-hp0 kt8-11
            q_vproj(deferred, big, "big", (8, 9, 10, 11))
            q_vproj(deferred, big, "big", (12, 13, 14, 15))
            q_qkproj(deferred, big, "big", [(3, 2)])  # K-hp1 kt8-11
            q_qkproj(deferred, big, "big", [(3, 3)])  # K-hp1 kt12-15
            for mt in range(8):  # qb0+qb1 outproj
                deferred.append(
                    lambda mt=mt: emit_outproj_mt(mt, big, "big")
                )
            q_qkproj(deferred, big, "big", [(0, 2), (1, 2)])  # qb2 Q

            for qb in (3, 2):
                for hp in range(2):
                    cx = cx_pool.tile(
                        [65, 2, 512], f32, tag="cx", name=f"cx{qb}_{hp}"
                    )
                    es_prev = None
                    for kt in range(4 * (qb + 1)):
                        es = emit_scores_exp(qb, hp, kt, big, "big")
                        if es_prev is not None:
                            emit_av(qb, hp, kt - 1, cx, es_prev)
                        if kt % 2 == 1 and len(deferred) >= 2:
                            deferred.popleft()()
                            deferred.popleft()()
                        es_prev = es
                    emit_av(qb, hp, 4 * (qb + 1) - 1, cx, es_prev)
                    emit_normalize(qb, hp, cx)
                for mt in range(4 * qb, 4 * qb + 4):
                    deferred.append(
                        lambda mt=mt: emit_outproj_mt(mt, big, "big")
                    )
            while deferred:
                deferred.popleft()()

            big_ctx.__exit__(None, None, None)
            cx_ctx.__exit__(None, None, None)

    nc.compile()
    return nc


def build_kernel(matmul_dtype=None, av_dtype=None):
    # single fp16 variant; dtype args accepted for harness compat
    if "k" not in _nc_cache:
        _nc_cache["k"] = _build()
    return _nc_cache["k"]


def make_in_maps(
    hidden_states, c_attn_w, c_attn_b, c_proj_w, c_proj_b,
    matmul_dtype=None, av_dtype=None,
):
    hidden_states = np.asarray(hidden_states, dtype=np.float32)
    c_attn_w = np.asarray(c_attn_w, dtype=np.float32)
    c_attn_b = np.asarray(c_attn_b, dtype=np.float32)
    c_proj_w = np.asarray(c_proj_w, dtype=np.float32)
    c_proj_b = np.asarray(c_proj_b, dtype=np.float32)

    in_maps = []
    for c in range(N_CORES):
        b, g = divmod(c, GROUPS)
        cs = slice(g * HSL, (g + 1) * HSL)
        wq = c_attn_w[:, g * HSL : (g + 1) * HSL]
        wk = c_attn_w[:, D + g * HSL : D + (g + 1) * HSL]
        wvs = c_attn_w[:, 2 * D + g * HSL : 2 * D + (g + 1) * HSL]
        bq = c_attn_b[g * HSL : (g + 1) * HSL]
        bk = c_attn_b[D + g * HSL : D + (g + 1) * HSL]
        bv = c_attn_b[2 * D + g * HSL : 2 * D + (g + 1) * HSL]
        wps = c_proj_w[cs, :]
        rr = bv.astype(np.float64) @ wps.astype(np.float64)
        if g == 0:
            rr = rr + c_proj_b
        in_maps.append(
            {
                "hsT": np.ascontiguousarray(
                    hidden_states[b].T.astype(np.float16)
                ),
                "wqk": np.ascontiguousarray(
                    np.concatenate(
                        [wq[:, :128], wk[:, :128], wq[:, 128:], wk[:, 128:]],
                        axis=1,
                    ).astype(np.float16)
                ),
                "wv": np.ascontiguousarray(wvs.astype(np.float16)),
                "wp": np.ascontiguousarray(wps.astype(np.float16)),
                "bqk": np.ascontiguousarray(
                    np.concatenate([bq[:128], bk[:128], bq[128:], bk[128:]])
                ),
                "_rrow": np.ascontiguousarray(rr.astype(np.float32)),
            }
        )
    return in_maps


def kernel(
    hidden_states,
    c_attn_w,
    c_attn_b,
    c_proj_w,
    c_proj_b,
    causal_mask=None,
    **_unused,
):
    from concourse.bass_utils import run_bass_kernel_spmd

    nc = build_kernel()
    in_maps = make_in_maps(
        hidden_states, c_attn_w, c_attn_b, c_proj_w, c_proj_b
    )
    rrows = [m.pop("_rrow") for m in in_maps]
    res = run_bass_kernel_spmd(nc, in_maps, list(range(N_CORES)))
    out = np.zeros((B, S, D), dtype=np.float32)
    for c in range(N_CORES):
        out[c // GROUPS] += res.results[c]["outp"].astype(np.float32)
        out[c // GROUPS] += rrows[c]
    return out
